# revision 1
# baseline (speedup 1.0000x reference)
"""DILATE loss (soft-DTW shape + temporal) on 8 Trainium2 NeuronCores.

Strategy (data-parallel, per the sharding hint): the 256 independent
(batch x channel) series are sharded 32 per core; each core runs its own
128x128 DP per series with series on SBUF partitions; the scalar loss is
reduced on the host.

Per-core algorithm (gamma=0.01 makes softmin ultra-sharp, so a min-plus
DP with a pseudo-posterior gradient matches the reference closely):
  D[i,j]   = (t_i - o_j)^2
  M[i,j]   = D[i,j] + min(M[i-1,j-1], M[i-1,j], M[i,j-1])        (forward Viterbi)
  num[i,j] = D[i,j] + min(num[i,j+1], num[i+1,j], num[i+1,j+1])  (suffix Viterbi)
  E*Omega  = exp(-lam*(M - D + num - M[N,N] + womg)),  womg = -ln(Omega)/lam
  vals     = M[N,N];   tl = sum_ij (E*Omega)[i,j]
  loss     = 0.5*sum(vals)/B + 0.5*sum(tl)/(B*T*T)

Each DP row is one TT-min + one tensor_tensor_scan (min,add) on the DVE;
the D build / suffix-term / exp / reduce phases are bulk ops overlapped
across GPSIMD / ACT / DVE by the Tile scheduler.
"""
import sys
if "/opt/trn_rl_repo" not in sys.path:
    sys.path.insert(0, "/opt/trn_rl_repo")
import numpy as np
from contextlib import ExitStack

import concourse.bass as bass
import concourse.bacc as bacc
import concourse.mybir as mybir
import concourse.tile as tile
from concourse.mybir import AluOpType, ActivationFunctionType

F32 = mybir.dt.float32
S = 32          # series per core
N = 128         # DP size (= T)
LAM = 100.0     # 1/gamma
BIG = 1e30
RS = N + 1
CH = 16      # row stride in the stores (value cols + 1 guard/boundary col)
N_CORES = 8


def ap(t, off, dims):
    base = t[:]
    return bass.AP(base.tensor, base.offset + off, [base.ap[0]] + dims)


def _build_kernel():
    nc = bacc.Bacc("TRN2", target_bir_lowering=False, debug=False)
    t_d = nc.dram_tensor("t", [S, N], F32, kind="ExternalInput")
    o_d = nc.dram_tensor("o", [S, N], F32, kind="ExternalInput")
    omg_d = nc.dram_tensor("omg", [S, N * N], F32, kind="ExternalInput")
    vals_d = nc.dram_tensor("vals", [S, 1], F32, kind="ExternalOutput")
    tl_d = nc.dram_tensor("tl", [S, 1], F32, kind="ExternalOutput")

    with tile.TileContext(nc) as tc, ExitStack() as ctx:
        pool = ctx.enter_context(tc.tile_pool(name="main", bufs=1))
        t_s = pool.tile([S, N], F32, tag="t_s")
        o_s = pool.tile([S, N], F32, tag="o_s")
        D_s = pool.tile([S, RS * N], F32, tag="D_s")        # D rows; Omega DMA'd in late
        MN_s = pool.tile([S, RS * (2 * N + 1)], F32, tag="MN_s")  # M rows 0..N, then num rows
        ent_s = pool.tile([S, 2 * N], F32, tag="ent_s")     # [0:N]=fwd ent, [N:2N]=bwd entb
        stgA = pool.tile([S, CH * N], F32, tag="stgA")
        vals_s = pool.tile([S, 1], F32, tag="vals_s")
        bias_s = pool.tile([S, 1], F32, tag="bias_s")
        tl_s = pool.tile([S, 1], F32, tag="tl_s")
        tlp_s = pool.tile([S, N // CH], F32, tag="tlp_s")

        nc.sync.dma_start(t_s[:], t_d.ap())
        nc.sync.dma_start(o_s[:], o_d.ap())

        NUMOFF = RS * (N + 1)

        def m_off(r):
            return r * RS

        def n_off(r):
            return NUMOFF + (r - 1) * RS

        # guards/boundaries
        nc.gpsimd.memset(ap(D_s, N, [[RS, N], [1, 1]]), BIG)          # D col guard
        nc.gpsimd.memset(ap(MN_s, NUMOFF + N, [[RS, N], [1, 1]]), BIG)  # num col guard
        nc.vector.memset(ap(MN_s, 0, [[RS, N + 1], [1, 1]]), BIG)     # M boundary col
        nc.vector.memset(ap(MN_s, 1, [[1, N]]), BIG)                  # M row 0
        nc.vector.memset(ap(MN_s, 0, [[1, 1]]), 0.0)

        # D build on DVE, interleaved order so both DPs start early:
        # c8 (bwd needs last rows first), then c1 (fwd), then c7, c2, ...
        order = []
        lo, hi = 0, N // CH - 1
        while hi >= lo:
            order.append(lo); lo += 1
            if lo <= hi:
                order.append(hi); hi -= 1
        DCH = 4
        order = []
        lo, hi = 0, N // DCH - 1
        while hi >= lo:
            order.append(lo); lo += 1
            if lo <= hi:
                order.append(hi); hi -= 1
        for c in order:
            c0 = c * DCH
            dch = ap(D_s, c0 * RS, [[RS, DCH], [1, N]])
            t_ch = ap(t_s, c0, [[1, DCH], [0, N]])
            o_ch = ap(o_s, 0, [[0, DCH], [1, N]])
            nc.gpsimd.tensor_tensor(dch, t_ch, o_ch, AluOpType.subtract)
            nc.scalar.activation(dch, dch, ActivationFunctionType.Square)

        # two independent DP chains (fwd ascending, suffix descending) —
        # kept separate so their instruction streams fill each other's
        # pipeline bubbles on the DVE.
        nc.gpsimd.memset(ap(ent_s, N, [[1, N]]), BIG)
        nc.gpsimd.memset(ap(ent_s, 2 * N - 1, [[1, 1]]), 0.0)
        for r in range(N, 0, -1):
            if r < N:
                nc.vector.tensor_tensor(
                    ap(ent_s, N, [[1, N]]),
                    ap(MN_s, n_off(r + 1), [[1, N]]),
                    ap(MN_s, n_off(r + 1) + 1, [[1, N]]),
                    AluOpType.min)
            nc.vector.tensor_tensor_scan(
                ap(MN_s, n_off(r) + N - 1, [[-1, N]]),
                ap(ent_s, 2 * N - 1, [[-1, N]]),
                ap(D_s, (r - 1) * RS + N - 1, [[-1, N]]),
                BIG, AluOpType.min, AluOpType.add)
        for r in range(1, N + 1):
            nc.vector.tensor_tensor(
                ap(ent_s, 0, [[1, N]]),
                ap(MN_s, m_off(r - 1), [[1, N]]), ap(MN_s, m_off(r - 1) + 1, [[1, N]]),
                AluOpType.min)
            nc.vector.tensor_tensor_scan(
                ap(MN_s, m_off(r) + 1, [[1, N]]), ap(ent_s, 0, [[1, N]]),
                ap(D_s, (r - 1) * RS, [[1, N]]),
                BIG, AluOpType.min, AluOpType.add)

        # vals = M[N,N]; bias = +lam*M[N,N]  (before womg is folded into M!)
        nc.vector.tensor_copy(vals_s[:], ap(MN_s, m_off(N) + N, [[1, 1]]))
        nc.vector.tensor_scalar(bias_s[:], vals_s[:], LAM, None, AluOpType.mult)

        # fold womg (= -ln(Omega)/lam) into M, chunk-staged through SBUF on
        # gpsimd -- runs during the row phase as fwd rows complete
        for ci in range(N // CH):
            c0 = ci * CH
            stg = stgA
            nc.sync.dma_start(stg[:], bass.AP(omg_d, c0 * N, [[N * N, S], [1, CH * N]]))
            mch = ap(MN_s, m_off(c0 + 1) + 1, [[RS, CH], [1, N]])
            nc.gpsimd.tensor_tensor(mch, mch, ap(stg, 0, [[N, CH], [1, N]]),
                                    AluOpType.add)

        # epilogue, chunked: arg = (M + womg) + num - D;
        # E*Omega = exp(-lam*arg + lam*MNN) with the chunk partial sum taken
        # directly from the ACT Exp's accum_out (no multiply, no DVE reduce)
        for ci in range(N // CH - 1, -1, -1):
            c0 = ci * CH
            eng = nc.gpsimd if ci <= 3 else nc.vector  # gps is free after its DP
            mch = ap(MN_s, m_off(c0 + 1) + 1, [[RS, CH], [1, N]])
            dch = ap(D_s, c0 * RS, [[RS, CH], [1, N]])
            nch = ap(MN_s, n_off(c0 + 1), [[RS, CH], [1, N]])
            eng.tensor_tensor(mch, mch, nch, AluOpType.add)        # + num
            eng.tensor_tensor(mch, mch, dch, AluOpType.subtract)   # - D
            nc.scalar.activation(nch, mch, ActivationFunctionType.Exp,
                                 bias=bias_s[:], scale=-LAM,
                                 accum_out=tlp_s[:, ci:ci + 1])
        nc.vector.tensor_reduce(tl_s[:], tlp_s[:], mybir.AxisListType.X, AluOpType.add)

        nc.sync.dma_start(vals_d.ap(), vals_s[:])
        nc.sync.dma_start(tl_d.ap(), tl_s[:])

    nc.compile()
    return nc



_NC_CACHE = None
_OMG_CACHE = None


def _get_nc():
    global _NC_CACHE
    if _NC_CACHE is None:
        _NC_CACHE = _build_kernel()
    return _NC_CACHE


def _womg():
    global _OMG_CACHE
    if _OMG_CACHE is None:
        idx = np.arange(1, N + 1, dtype=np.float64)
        om2d = ((idx[:, None] - idx[None, :]) ** 2).reshape(N * N)
        w = np.where(om2d == 0.0, BIG, -np.log(np.maximum(om2d, 1e-30)) / LAM)
        _OMG_CACHE = np.ascontiguousarray(
            np.broadcast_to(w.astype(np.float32), (S, N * N)))
    return _OMG_CACHE


_EXEC_CACHE = None


def _get_exec():
    """Build the sharded jitted executable once (mirrors bass2jax's
    run_bass_via_pjrt multi-core path) and keep the big constant omg input
    resident on the devices."""
    global _EXEC_CACHE
    if _EXEC_CACHE is not None:
        return _EXEC_CACHE
    import jax
    import concourse.mybir as _mybir
    from jax.sharding import Mesh, PartitionSpec, NamedSharding
    from jax.experimental.shard_map import shard_map
    from concourse.bass2jax import (
        _bass_exec_p, install_neuronx_cc_hook, partition_id_tensor)

    nc = _get_nc()
    install_neuronx_cc_hook()
    partition_name = nc.partition_id_tensor.name if nc.partition_id_tensor else None
    in_names, out_names, out_avals, zero_outs = [], [], [], []
    for alloc in nc.m.functions[0].allocations:
        if not isinstance(alloc, _mybir.MemoryLocationSet):
            continue
        name = alloc.memorylocations[0].name
        if alloc.kind == "ExternalInput":
            if name != partition_name:
                in_names.append(name)
        elif alloc.kind == "ExternalOutput":
            shape = tuple(alloc.tensor_shape)
            dtype = _mybir.dt.np(alloc.dtype)
            out_names.append(name)
            out_avals.append(jax.core.ShapedArray(shape, dtype))
            zero_outs.append(np.zeros(shape, dtype))
    n_params = len(in_names)
    all_in_names = list(in_names) + list(out_names)
    if partition_name is not None:
        all_in_names.append(partition_name)
    donate = tuple(range(n_params, n_params + len(out_names)))

    def _body(*args):
        operands = list(args)
        if partition_name is not None:
            operands.append(partition_id_tensor())
        return tuple(_bass_exec_p.bind(
            *operands,
            out_avals=tuple(out_avals),
            in_names=tuple(all_in_names),
            out_names=tuple(out_names),
            lowering_input_output_aliases=(),
            sim_require_finite=True,
            sim_require_nnan=True,
            nc=nc,
        ))

    devices = jax.devices()[:N_CORES]
    mesh = Mesh(np.asarray(devices), ("core",))
    in_specs = (PartitionSpec("core"),) * (n_params + len(out_names))
    out_specs = (PartitionSpec("core"),) * len(out_names)
    sharded = jax.jit(
        shard_map(_body, mesh=mesh, in_specs=in_specs, out_specs=out_specs,
                  check_rep=False),
        donate_argnums=donate, keep_unused=True)
    shard = NamedSharding(mesh, PartitionSpec("core"))
    omg_dev = jax.device_put(
        np.concatenate([_womg()] * N_CORES, axis=0), shard)
    _EXEC_CACHE = (sharded, in_names, out_names, zero_outs, shard, omg_dev)
    return _EXEC_CACHE


def kernel(outputs, targets):
    """outputs, targets: [64, 128, 4] float32 -> scalar float32 loss."""
    sharded, in_names, out_names, zero_outs, shard, omg_dev = _get_exec()
    outputs = np.asarray(outputs, np.float32)
    targets = np.asarray(targets, np.float32)
    B, T, C = outputs.shape
    t = np.ascontiguousarray(np.transpose(targets, (0, 2, 1)).reshape(B * C, T))
    o = np.ascontiguousarray(np.transpose(outputs, (0, 2, 1)).reshape(B * C, T))
    by_name = {"t": t, "o": o, "omg": omg_dev}
    concat_in = [by_name[name] for name in in_names]
    concat_zeros = [
        np.zeros((N_CORES * z.shape[0], *z.shape[1:]), z.dtype) for z in zero_outs
    ]
    out_arrs = sharded(*concat_in, *concat_zeros)
    outs = {name: np.asarray(out_arrs[i]) for i, name in enumerate(out_names)}
    vals = outs["vals"][:, 0]
    tl = outs["tl"][:, 0]
    loss = 0.5 * (vals.sum(dtype=np.float64) / B) + \
           0.5 * (tl.sum(dtype=np.float64) / (B * T * T))
    return np.float32(loss)



# revision 2
# speedup vs baseline: 1.3620x; 1.3620x over previous
"""DILATE loss (soft-DTW shape + temporal) on 8 Trainium2 NeuronCores.

Strategy: central finite difference. gamma=0.01 makes the soft-DTW
effectively a hard min-plus (Viterbi) DP, and the temporal term is
  sum(E * Omega) = d/d(eps) softdtw(D + eps*Omega)  at eps=0
since E = d(softdtw)/dD. So each series is solved TWICE — once on
D + eps*Omega and once on D - eps*Omega — and the host combines:
  vals = (A + B) / 2          (shape term)
  sum(E*Omega) = (A - B)/(2 eps)   (temporal term)
This removes the whole backward/suffix DP, the posterior epilogue
(+num, -D, exp/accumulate) and the -ln(Omega) fold of the previous
version; only ONE forward DP chain remains.

Layout per core: 32 series x {+eps, -eps} = 64 SBUF partitions; per-op
cost depends only on free-dim size, so the doubling is time-free.
  ACT : D rows fused build  Square(-o_j + t_i)  (bias = per-partition t_i)
  Pool: D += (+-eps)Omega   via tensor_tensor_scan(bypass, add) on flat
        chunks (the +- sign is baked into the per-half V constant)
  DVE : 128 x (TT-min + scan(min,add)) forward DP  — the critical path
"""
import sys
if "/opt/trn_rl_repo" not in sys.path:
    sys.path.insert(0, "/opt/trn_rl_repo")
import numpy as np
from contextlib import ExitStack

import concourse.bass as bass
import concourse.bacc as bacc
import concourse.mybir as mybir
import concourse.tile as tile
from concourse.mybir import AluOpType, ActivationFunctionType

F32 = mybir.dt.float32
F16 = mybir.dt.float16
S = 32          # series per core
S2 = 64         # partitions: series x {+eps, -eps}
N = 128         # DP size (= T)
RS = N + 1      # M-table row stride (col 0 = boundary)
BIG = 1e30
EPS = 3e-5      # FD step on the Omega perturbation
N_CORES = 8


def ap(t, off, dims):
    base = t[:]
    return bass.AP(base.tensor, base.offset + off, [base.ap[0]] + dims)


def _build_kernel():
    nc = bacc.Bacc("TRN2", target_bir_lowering=False, debug=False)
    t_d = nc.dram_tensor("t", [S2, N], F32, kind="ExternalInput")
    o_d = nc.dram_tensor("o", [S2, N], F32, kind="ExternalInput")
    v_d = nc.dram_tensor("v", [S2, N * N], F16, kind="ExternalInput")
    vals_d = nc.dram_tensor("vals", [S2, 1], F32, kind="ExternalOutput")

    with tile.TileContext(nc) as tc, ExitStack() as ctx:
        pool = ctx.enter_context(tc.tile_pool(name="main", bufs=1))
        t_s = pool.tile([S2, N], F32, tag="t_s")
        o_s = pool.tile([S2, N], F32, tag="o_s")
        v_s = pool.tile([S2, N * N], F16, tag="v_s")
        D_s = pool.tile([S2, N * N], F32, tag="D_s")
        M_s = pool.tile([S2, RS * RS], F32, tag="M_s")
        ent_s = pool.tile([S2, N], F32, tag="ent_s")
        vals_s = pool.tile([S2, 1], F32, tag="vals_s")

        nc.sync.dma_start(t_s[:], t_d.ap())
        nc.sync.dma_start(o_s[:], o_d.ap())
        # eps*Omega chunks: first ones small so the build pipeline starts fast
        vch = [4, 12, 16, 16, 16, 16, 16, 16, 16]
        r0 = 0
        for cn in vch:
            nc.sync.dma_start(
                ap(v_s, r0 * N, [[1, cn * N]]),
                bass.AP(v_d, r0 * N, [[N * N, S2], [1, cn * N]]))
            r0 += cn

        # M boundary: row 0 = BIG except M[0,0] = 0; col 0 of rows 1..N = BIG
        nc.gpsimd.memset(ap(M_s, 0, [[1, RS]]), BIG)
        nc.gpsimd.memset(ap(M_s, 0, [[1, 1]]), 0.0)
        nc.gpsimd.memset(ap(M_s, RS, [[RS, N], [1, 1]]), BIG)

        def m_off(r):
            return r * RS

        # D build: ACT fuses (t_i - o_j)^2 per row; Pool folds +-eps*Omega in
        # flat scan(bypass, add) chunks right behind it.
        row_chunks = [2, 2, 4, 8, 16, 16, 16, 16, 16, 16, 16]
        r0 = 0
        for cn in row_chunks:
            for i in range(r0, r0 + cn):
                nc.scalar.activation(
                    ap(D_s, i * N, [[1, N]]), o_s[:],
                    ActivationFunctionType.Square,
                    bias=t_s[:, i:i + 1], scale=-1.0)
            dch = ap(D_s, r0 * N, [[1, cn * N]])
            nc.gpsimd.tensor_tensor_scan(
                dch, dch, ap(v_s, r0 * N, [[1, cn * N]]),
                0.0, AluOpType.bypass, AluOpType.add)
            r0 += cn

        # forward min-plus DP: M[r,j] = D[r,j] + min(M[r-1,j-1], M[r-1,j], M[r,j-1])
        for r in range(1, N + 1):
            nc.vector.tensor_tensor(
                ent_s[:],
                ap(M_s, m_off(r - 1), [[1, N]]),
                ap(M_s, m_off(r - 1) + 1, [[1, N]]),
                AluOpType.min)
            nc.vector.tensor_tensor_scan(
                ap(M_s, m_off(r) + 1, [[1, N]]),
                ent_s[:],
                ap(D_s, (r - 1) * N, [[1, N]]),
                BIG, AluOpType.min, AluOpType.add)

        nc.vector.tensor_copy(vals_s[:], ap(M_s, m_off(N) + N, [[1, 1]]))
        nc.sync.dma_start(vals_d.ap(), vals_s[:])

    nc.compile()
    return nc


_NC_CACHE = None


def _get_nc():
    global _NC_CACHE
    if _NC_CACHE is None:
        _NC_CACHE = _build_kernel()
    return _NC_CACHE


def _v_const():
    """[S2, N*N] fp16: rows 0..31 = +eps*Omega, rows 32..63 = -eps*Omega."""
    idx = np.arange(N, dtype=np.float64)
    om = ((idx[:, None] - idx[None, :]) ** 2).reshape(-1)
    v = (EPS * om).astype(np.float16)
    return np.concatenate([
        np.broadcast_to(v, (S, N * N)),
        np.broadcast_to(-v, (S, N * N)),
    ]).astype(np.float16)


_EXEC_CACHE = None


def _get_exec():
    """Build the sharded jitted executable once (mirrors bass2jax's
    run_bass_via_pjrt multi-core path) and keep the constant v input
    resident on the devices."""
    global _EXEC_CACHE
    if _EXEC_CACHE is not None:
        return _EXEC_CACHE
    import jax
    import concourse.mybir as _mybir
    from jax.sharding import Mesh, PartitionSpec, NamedSharding
    from jax.experimental.shard_map import shard_map
    from concourse.bass2jax import (
        _bass_exec_p, install_neuronx_cc_hook, partition_id_tensor)

    nc = _get_nc()
    install_neuronx_cc_hook()
    partition_name = nc.partition_id_tensor.name if nc.partition_id_tensor else None
    in_names, out_names, out_avals, zero_outs = [], [], [], []
    for alloc in nc.m.functions[0].allocations:
        if not isinstance(alloc, _mybir.MemoryLocationSet):
            continue
        name = alloc.memorylocations[0].name
        if alloc.kind == "ExternalInput":
            if name != partition_name:
                in_names.append(name)
        elif alloc.kind == "ExternalOutput":
            shape = tuple(alloc.tensor_shape)
            dtype = _mybir.dt.np(alloc.dtype)
            out_names.append(name)
            out_avals.append(jax.core.ShapedArray(shape, dtype))
            zero_outs.append(np.zeros(shape, dtype))
    n_params = len(in_names)
    all_in_names = list(in_names) + list(out_names)
    if partition_name is not None:
        all_in_names.append(partition_name)
    donate = tuple(range(n_params, n_params + len(out_names)))

    def _body(*args):
        operands = list(args)
        if partition_name is not None:
            operands.append(partition_id_tensor())
        return tuple(_bass_exec_p.bind(
            *operands,
            out_avals=tuple(out_avals),
            in_names=tuple(all_in_names),
            out_names=tuple(out_names),
            lowering_input_output_aliases=(),
            sim_require_finite=True,
            sim_require_nnan=True,
            nc=nc,
        ))

    devices = jax.devices()[:N_CORES]
    mesh = Mesh(np.asarray(devices), ("core",))
    in_specs = (PartitionSpec("core"),) * (n_params + len(out_names))
    out_specs = (PartitionSpec("core"),) * len(out_names)
    sharded = jax.jit(
        shard_map(_body, mesh=mesh, in_specs=in_specs, out_specs=out_specs,
                  check_rep=False),
        donate_argnums=donate, keep_unused=True)
    shard = NamedSharding(mesh, PartitionSpec("core"))
    v_dev = jax.device_put(
        np.concatenate([_v_const()] * N_CORES, axis=0), shard)
    _EXEC_CACHE = (sharded, in_names, out_names, zero_outs, shard, v_dev)
    return _EXEC_CACHE


def kernel(outputs, targets):
    """outputs, targets: [64, 128, 4] float32 -> scalar float32 loss."""
    sharded, in_names, out_names, zero_outs, shard, v_dev = _get_exec()
    outputs = np.asarray(outputs, np.float32)
    targets = np.asarray(targets, np.float32)
    B, T, C = outputs.shape
    t = np.transpose(targets, (0, 2, 1)).reshape(N_CORES, S, T)
    o = np.transpose(outputs, (0, 2, 1)).reshape(N_CORES, S, T)
    # per core: 64 partitions = [32 series | same 32 series again]
    t2 = np.ascontiguousarray(
        np.concatenate([t, t], axis=1).reshape(N_CORES * S2, T))
    o2 = np.ascontiguousarray(
        np.concatenate([o, o], axis=1).reshape(N_CORES * S2, T))
    by_name = {"t": t2, "o": o2, "v": v_dev}
    concat_in = [by_name[name] for name in in_names]
    concat_zeros = [
        np.zeros((N_CORES * z.shape[0], *z.shape[1:]), z.dtype) for z in zero_outs
    ]
    out_arrs = sharded(*concat_in, *concat_zeros)
    outs = {name: np.asarray(out_arrs[i]) for i, name in enumerate(out_names)}
    vv = outs["vals"].reshape(N_CORES, 2, S).astype(np.float64)
    A, Bm = vv[:, 0, :], vv[:, 1, :]
    vals = (A + Bm) / 2.0
    s_fd = (A - Bm) / (2.0 * EPS)
    loss = 0.5 * (vals.sum() / B) + 0.5 * (s_fd.sum() / (B * T * T))
    return np.float32(loss)


# revision 3
# speedup vs baseline: 2.0460x; 1.5022x over previous
"""DILATE loss (soft-DTW shape + temporal) on 8 Trainium2 NeuronCores.

Strategy: central finite difference. gamma=0.01 makes the soft-DTW
effectively a hard min-plus (Viterbi) DP, and the temporal term is
  sum(E * Omega) = d/d(eps) softdtw(D + eps*Omega)  at eps=0
since E = d(softdtw)/dD. So each series is solved TWICE — once on
D + eps*Omega and once on D - eps*Omega — and the host combines:
  vals = (A + B) / 2          (shape term)
  sum(E*Omega) = (A - B)/(2 eps)   (temporal term)
This removes the whole backward/suffix DP, the posterior epilogue
(+num, -D, exp/accumulate) and the -ln(Omega) fold of the previous
version; only ONE forward DP chain remains.

Layout per core: 32 series x {+eps, -eps} = 64 SBUF partitions; per-op
cost depends only on free-dim size, so the doubling is time-free.
  ACT : D rows fused build  Square(-o_j + t_i)  (bias = per-partition t_i)
  Pool: D += (+-eps)Omega   via tensor_tensor_scan(bypass, add) on flat
        chunks (the +- sign is baked into the per-half V constant)
  DVE : 128 x (TT-min + scan(min,add)) forward DP  — the critical path
"""
import sys
if "/opt/trn_rl_repo" not in sys.path:
    sys.path.insert(0, "/opt/trn_rl_repo")
import numpy as np
from contextlib import ExitStack

import concourse.bass as bass
import concourse.bacc as bacc
import concourse.mybir as mybir
import concourse.tile as tile
from concourse.mybir import AluOpType, ActivationFunctionType

F32 = mybir.dt.float32
F16 = mybir.dt.float16
S = 32          # series per core
S2 = 64         # partitions: series x {+eps, -eps}
N = 128         # DP size (= T)
RS = N + 1      # M-table row stride (col 0 = boundary)
BIG = 1e30
EPS = 3e-5      # FD step on the Omega perturbation
N_CORES = 8


def ap(t, off, dims):
    base = t[:]
    return bass.AP(base.tensor, base.offset + off, [base.ap[0]] + dims)


def _build_kernel():
    nc = bacc.Bacc("TRN2", target_bir_lowering=False, debug=False)
    t_d = nc.dram_tensor("t", [S2, N], F32, kind="ExternalInput")
    o_d = nc.dram_tensor("o", [S2, N], F32, kind="ExternalInput")
    v_d = nc.dram_tensor("v", [S2, N * N], F16, kind="ExternalInput")
    vals_d = nc.dram_tensor("vals", [S2, 1], F32, kind="ExternalOutput")

    with tile.TileContext(nc) as tc, ExitStack() as ctx:
        pool = ctx.enter_context(tc.tile_pool(name="main", bufs=1))
        t_s = pool.tile([S2, N], F32, tag="t_s")
        o_s = pool.tile([S2, N], F32, tag="o_s")
        v_s = pool.tile([S2, N * N], F16, tag="v_s")
        D_s = pool.tile([S2, N * N], F32, tag="D_s")
        M_s = pool.tile([S2, RS * RS], F32, tag="M_s")
        ent_s = pool.tile([S2, N], F32, tag="ent_s")
        vals_s = pool.tile([S2, 1], F32, tag="vals_s")

        nc.sync.dma_start(t_s[:], t_d.ap())
        nc.sync.dma_start(o_s[:], o_d.ap())
        # eps*Omega chunks: first ones small so the build pipeline starts fast
        vch = [4, 12, 16, 16, 16, 16, 16, 16, 16]
        r0 = 0
        for cn in vch:
            nc.sync.dma_start(
                ap(v_s, r0 * N, [[1, cn * N]]),
                bass.AP(v_d, r0 * N, [[N * N, S2], [1, cn * N]]))
            r0 += cn

        # M boundary: row 0 = BIG except M[0,0] = 0; col 0 of rows 1..N = BIG
        nc.gpsimd.memset(ap(M_s, 0, [[1, RS]]), BIG)
        nc.gpsimd.memset(ap(M_s, 0, [[1, 1]]), 0.0)
        nc.gpsimd.memset(ap(M_s, RS, [[RS, N], [1, 1]]), BIG)

        def m_off(r):
            return r * RS

        # D build: ACT fuses (t_i - o_j)^2 per row; Pool folds +-eps*Omega in
        # flat scan(bypass, add) chunks right behind it.
        row_chunks = [2, 2, 4, 8, 16, 16, 16, 16, 16, 16, 16]
        r0 = 0
        for cn in row_chunks:
            for i in range(r0, r0 + cn):
                nc.scalar.activation(
                    ap(D_s, i * N, [[1, N]]), o_s[:],
                    ActivationFunctionType.Square,
                    bias=t_s[:, i:i + 1], scale=-1.0)
            dch = ap(D_s, r0 * N, [[1, cn * N]])
            nc.gpsimd.tensor_tensor_scan(
                dch, dch, ap(v_s, r0 * N, [[1, cn * N]]),
                0.0, AluOpType.bypass, AluOpType.add)
            r0 += cn

        # forward min-plus DP: M[r,j] = D[r,j] + min(M[r-1,j-1], M[r-1,j], M[r,j-1])
        dp_insts = []
        for r in range(1, N + 1):
            dp_insts.append(nc.vector.tensor_tensor(
                ent_s[:],
                ap(M_s, m_off(r - 1), [[1, N]]),
                ap(M_s, m_off(r - 1) + 1, [[1, N]]),
                AluOpType.min))
            dp_insts.append(nc.vector.tensor_tensor_scan(
                ap(M_s, m_off(r) + 1, [[1, N]]),
                ent_s[:],
                ap(D_s, (r - 1) * N, [[1, N]]),
                BIG, AluOpType.min, AluOpType.add))

        # The DP is one serial chain of same-engine (DVE) ops with
        # ascending same-shape access; engine in-order execution plus the
        # 128-cycle op length covers the SBUF write-ack pipeline, so the
        # intra-chain edges don't need runtime semaphores. Relax them to
        # nosync (queue-order) edges — without this every row pays two
        # ~90ns sem round-trips plus a SEQ-blocking EventSemaphore wait
        # (~290ns/row, +37us on the critical path).
        import bass_rust as _br
        _NOSYNC = _br.DependencyInfo.NO_SYNC_ONLY
        dp_names = {bi.ins.name for bi in dp_insts}
        for bi in dp_insts:
            inst = bi.ins
            for name, info in inst.dependency_edges():
                if name in dp_names:
                    inst.remove_dependency(name)
                    inst.add_dependency(name, _NOSYNC)

        nc.vector.tensor_copy(vals_s[:], ap(M_s, m_off(N) + N, [[1, 1]]))
        nc.sync.dma_start(vals_d.ap(), vals_s[:])

    nc.compile()
    return nc


_NC_CACHE = None


def _get_nc():
    global _NC_CACHE
    if _NC_CACHE is None:
        _NC_CACHE = _build_kernel()
    return _NC_CACHE


def _v_const():
    """[S2, N*N] fp16: rows 0..31 = +eps*Omega, rows 32..63 = -eps*Omega."""
    idx = np.arange(N, dtype=np.float64)
    om = ((idx[:, None] - idx[None, :]) ** 2).reshape(-1)
    v = (EPS * om).astype(np.float16)
    return np.concatenate([
        np.broadcast_to(v, (S, N * N)),
        np.broadcast_to(-v, (S, N * N)),
    ]).astype(np.float16)


_EXEC_CACHE = None


def _get_exec():
    """Build the sharded jitted executable once (mirrors bass2jax's
    run_bass_via_pjrt multi-core path) and keep the constant v input
    resident on the devices."""
    global _EXEC_CACHE
    if _EXEC_CACHE is not None:
        return _EXEC_CACHE
    import jax
    import concourse.mybir as _mybir
    from jax.sharding import Mesh, PartitionSpec, NamedSharding
    from jax.experimental.shard_map import shard_map
    from concourse.bass2jax import (
        _bass_exec_p, install_neuronx_cc_hook, partition_id_tensor)

    nc = _get_nc()
    install_neuronx_cc_hook()
    partition_name = nc.partition_id_tensor.name if nc.partition_id_tensor else None
    in_names, out_names, out_avals, zero_outs = [], [], [], []
    for alloc in nc.m.functions[0].allocations:
        if not isinstance(alloc, _mybir.MemoryLocationSet):
            continue
        name = alloc.memorylocations[0].name
        if alloc.kind == "ExternalInput":
            if name != partition_name:
                in_names.append(name)
        elif alloc.kind == "ExternalOutput":
            shape = tuple(alloc.tensor_shape)
            dtype = _mybir.dt.np(alloc.dtype)
            out_names.append(name)
            out_avals.append(jax.core.ShapedArray(shape, dtype))
            zero_outs.append(np.zeros(shape, dtype))
    n_params = len(in_names)
    all_in_names = list(in_names) + list(out_names)
    if partition_name is not None:
        all_in_names.append(partition_name)
    donate = tuple(range(n_params, n_params + len(out_names)))

    def _body(*args):
        operands = list(args)
        if partition_name is not None:
            operands.append(partition_id_tensor())
        return tuple(_bass_exec_p.bind(
            *operands,
            out_avals=tuple(out_avals),
            in_names=tuple(all_in_names),
            out_names=tuple(out_names),
            lowering_input_output_aliases=(),
            sim_require_finite=True,
            sim_require_nnan=True,
            nc=nc,
        ))

    devices = jax.devices()[:N_CORES]
    mesh = Mesh(np.asarray(devices), ("core",))
    in_specs = (PartitionSpec("core"),) * (n_params + len(out_names))
    out_specs = (PartitionSpec("core"),) * len(out_names)
    sharded = jax.jit(
        shard_map(_body, mesh=mesh, in_specs=in_specs, out_specs=out_specs,
                  check_rep=False),
        donate_argnums=donate, keep_unused=True)
    shard = NamedSharding(mesh, PartitionSpec("core"))
    v_dev = jax.device_put(
        np.concatenate([_v_const()] * N_CORES, axis=0), shard)
    _EXEC_CACHE = (sharded, in_names, out_names, zero_outs, shard, v_dev)
    return _EXEC_CACHE


def kernel(outputs, targets):
    """outputs, targets: [64, 128, 4] float32 -> scalar float32 loss."""
    sharded, in_names, out_names, zero_outs, shard, v_dev = _get_exec()
    outputs = np.asarray(outputs, np.float32)
    targets = np.asarray(targets, np.float32)
    B, T, C = outputs.shape
    t = np.transpose(targets, (0, 2, 1)).reshape(N_CORES, S, T)
    o = np.transpose(outputs, (0, 2, 1)).reshape(N_CORES, S, T)
    # per core: 64 partitions = [32 series | same 32 series again]
    t2 = np.ascontiguousarray(
        np.concatenate([t, t], axis=1).reshape(N_CORES * S2, T))
    o2 = np.ascontiguousarray(
        np.concatenate([o, o], axis=1).reshape(N_CORES * S2, T))
    by_name = {"t": t2, "o": o2, "v": v_dev}
    concat_in = [by_name[name] for name in in_names]
    concat_zeros = [
        np.zeros((N_CORES * z.shape[0], *z.shape[1:]), z.dtype) for z in zero_outs
    ]
    out_arrs = sharded(*concat_in, *concat_zeros)
    outs = {name: np.asarray(out_arrs[i]) for i, name in enumerate(out_names)}
    vv = outs["vals"].reshape(N_CORES, 2, S).astype(np.float64)
    A, Bm = vv[:, 0, :], vv[:, 1, :]
    vals = (A + Bm) / 2.0
    s_fd = (A - Bm) / (2.0 * EPS)
    loss = 0.5 * (vals.sum() / B) + 0.5 * (s_fd.sum() / (B * T * T))
    return np.float32(loss)


# revision 6
# speedup vs baseline: 2.1435x; 1.0476x over previous
"""DILATE loss (soft-DTW shape + temporal) on 8 Trainium2 NeuronCores.

Strategy: central finite difference. gamma=0.01 makes the soft-DTW
effectively a hard min-plus (Viterbi) DP, and the temporal term is
  sum(E * Omega) = d/d(eps) softdtw(D + eps*Omega)  at eps=0
since E = d(softdtw)/dD. So each series is solved TWICE — once on
D + eps*Omega and once on D - eps*Omega — and the host combines:
  vals = (A + B) / 2          (shape term)
  sum(E*Omega) = (A - B)/(2 eps)   (temporal term)
This removes the whole backward/suffix DP, the posterior epilogue
(+num, -D, exp/accumulate) and the -ln(Omega) fold of the previous
version; only ONE forward DP chain remains.

Layout per core: 32 series x {+eps, -eps} = 64 SBUF partitions; per-op
cost depends only on free-dim size, so the doubling is time-free.
  ACT : D rows fused build  Square(-o_j + t_i)  (bias = per-partition t_i)
  Pool: D += (+-eps)Omega   via tensor_tensor_scan(bypass, add) on flat
        chunks (the +- sign is baked into the per-half V constant)
  DVE : 128 x (TT-min + scan(min,add)) forward DP  — the critical path
"""
import sys
if "/opt/trn_rl_repo" not in sys.path:
    sys.path.insert(0, "/opt/trn_rl_repo")
import numpy as np
from contextlib import ExitStack

import concourse.bass as bass
import concourse.bacc as bacc
import concourse.mybir as mybir
import concourse.tile as tile
from concourse.mybir import AluOpType, ActivationFunctionType

F32 = mybir.dt.float32
F16 = mybir.dt.float16
S = 32          # series per core
S2 = 64         # partitions: series x {+eps, -eps}
N = 128         # DP size (= T)
RS = N + 1      # M-table row stride (col 0 = boundary)
BIG = 1e30
EPS = 3e-5      # FD step on the Omega perturbation
N_CORES = 8


def ap(t, off, dims):
    base = t[:]
    return bass.AP(base.tensor, base.offset + off, [base.ap[0]] + dims)


def _build_kernel():
    nc = bacc.Bacc("TRN2", target_bir_lowering=False, debug=False)
    t_d = nc.dram_tensor("t", [S2, N], F32, kind="ExternalInput")
    o_d = nc.dram_tensor("o", [S2, N], F32, kind="ExternalInput")
    v_d = nc.dram_tensor("v", [S2, N * N], F16, kind="ExternalInput")
    vals_d = nc.dram_tensor("vals", [S2, 1], F32, kind="ExternalOutput")

    with tile.TileContext(nc) as tc, ExitStack() as ctx:
        pool = ctx.enter_context(tc.tile_pool(name="main", bufs=1))
        t_s = pool.tile([S2, N], F32, tag="t_s")
        o_s = pool.tile([S2, N], F32, tag="o_s")
        v_s = pool.tile([S2, N * N], F16, tag="v_s")
        D_s = pool.tile([S2, N * N], F32, tag="D_s")
        M_s = pool.tile([S2, RS * RS], F32, tag="M_s")
        ent_s = pool.tile([S2, N], F32, tag="ent_s")
        vals_s = pool.tile([S2, 1], F32, tag="vals_s")

        # dummy activation on scratch: hoists the Square table load (1.28us)
        # to t=0 so it overlaps the input DMAs instead of stalling row 1
        nc.scalar.activation(vals_s[:], vals_s[:],
                             ActivationFunctionType.Square)

        nc.sync.dma_start(t_s[:], t_d.ap())
        nc.sync.dma_start(o_s[:], o_d.ap())
        # eps*Omega chunks: first ones small so the build pipeline starts fast
        vch = [2, 2, 4, 8, 16, 32, 64]
        r0 = 0
        for cn in vch:
            nc.sync.dma_start(
                ap(v_s, r0 * N, [[1, cn * N]]),
                bass.AP(v_d, r0 * N, [[N * N, S2], [1, cn * N]]))
            r0 += cn

        # M boundary: row 0 = BIG except M[0,0] = 0; col 0 of rows 1..N = BIG
        nc.gpsimd.memset(ap(M_s, 0, [[1, RS]]), BIG)
        nc.gpsimd.memset(ap(M_s, 0, [[1, 1]]), 0.0)
        nc.gpsimd.memset(ap(M_s, RS, [[RS, N], [1, 1]]), BIG)

        def m_off(r):
            return r * RS

        # D build: ACT fuses (t_i - o_j)^2 per row; Pool folds +-eps*Omega in
        # flat scan(bypass, add) chunks right behind it.
        row_chunks = [1, 1, 2, 2, 2, 4, 4, 8, 8, 16, 16, 16, 16, 16, 16]
        r0 = 0
        for cn in row_chunks:
            for i in range(r0, r0 + cn):
                nc.scalar.activation(
                    ap(D_s, i * N, [[1, N]]), o_s[:],
                    ActivationFunctionType.Square,
                    bias=t_s[:, i:i + 1], scale=-1.0)
            dch = ap(D_s, r0 * N, [[1, cn * N]])
            nc.gpsimd.tensor_tensor_scan(
                dch, dch, ap(v_s, r0 * N, [[1, cn * N]]),
                0.0, AluOpType.bypass, AluOpType.add)
            r0 += cn

        # forward min-plus DP: M[r,j] = D[r,j] + min(M[r-1,j-1], M[r-1,j], M[r,j-1])
        dp_insts = []
        for r in range(1, N + 1):
            dp_insts.append(nc.vector.tensor_tensor(
                ent_s[:],
                ap(M_s, m_off(r - 1), [[1, N]]),
                ap(M_s, m_off(r - 1) + 1, [[1, N]]),
                AluOpType.min))
            dp_insts.append(nc.vector.tensor_tensor_scan(
                ap(M_s, m_off(r) + 1, [[1, N]]),
                ent_s[:],
                ap(D_s, (r - 1) * N, [[1, N]]),
                BIG, AluOpType.min, AluOpType.add))

        # The DP is one serial chain of same-engine (DVE) ops with
        # ascending same-shape access; engine in-order execution plus the
        # 128-cycle op length covers the SBUF write-ack pipeline, so the
        # intra-chain edges don't need runtime semaphores. Relax them to
        # nosync (queue-order) edges — without this every row pays two
        # ~90ns sem round-trips plus a SEQ-blocking EventSemaphore wait
        # (~290ns/row, +37us on the critical path).
        import bass_rust as _br
        _NOSYNC = _br.DependencyInfo.NO_SYNC_ONLY
        dp_names = {bi.ins.name for bi in dp_insts}
        for bi in dp_insts:
            inst = bi.ins
            for name, info in inst.dependency_edges():
                if name in dp_names:
                    inst.remove_dependency(name)
                    inst.add_dependency(name, _NOSYNC)

        nc.sync.dma_start(vals_d.ap(), ap(M_s, m_off(N) + N, [[1, 1]]))

    nc.compile()
    return nc


_NC_CACHE = None


def _get_nc():
    global _NC_CACHE
    if _NC_CACHE is None:
        _NC_CACHE = _build_kernel()
    return _NC_CACHE


def _v_const():
    """[S2, N*N] fp16: rows 0..31 = +eps*Omega, rows 32..63 = -eps*Omega."""
    idx = np.arange(N, dtype=np.float64)
    om = ((idx[:, None] - idx[None, :]) ** 2).reshape(-1)
    v = (EPS * om).astype(np.float16)
    return np.concatenate([
        np.broadcast_to(v, (S, N * N)),
        np.broadcast_to(-v, (S, N * N)),
    ]).astype(np.float16)


_EXEC_CACHE = None


def _get_exec():
    """Build the sharded jitted executable once (mirrors bass2jax's
    run_bass_via_pjrt multi-core path) and keep the constant v input
    resident on the devices."""
    global _EXEC_CACHE
    if _EXEC_CACHE is not None:
        return _EXEC_CACHE
    import jax
    import concourse.mybir as _mybir
    from jax.sharding import Mesh, PartitionSpec, NamedSharding
    from jax.experimental.shard_map import shard_map
    from concourse.bass2jax import (
        _bass_exec_p, install_neuronx_cc_hook, partition_id_tensor)

    nc = _get_nc()
    install_neuronx_cc_hook()
    partition_name = nc.partition_id_tensor.name if nc.partition_id_tensor else None
    in_names, out_names, out_avals, zero_outs = [], [], [], []
    for alloc in nc.m.functions[0].allocations:
        if not isinstance(alloc, _mybir.MemoryLocationSet):
            continue
        name = alloc.memorylocations[0].name
        if alloc.kind == "ExternalInput":
            if name != partition_name:
                in_names.append(name)
        elif alloc.kind == "ExternalOutput":
            shape = tuple(alloc.tensor_shape)
            dtype = _mybir.dt.np(alloc.dtype)
            out_names.append(name)
            out_avals.append(jax.core.ShapedArray(shape, dtype))
            zero_outs.append(np.zeros(shape, dtype))
    n_params = len(in_names)
    all_in_names = list(in_names) + list(out_names)
    if partition_name is not None:
        all_in_names.append(partition_name)
    donate = tuple(range(n_params, n_params + len(out_names)))

    def _body(*args):
        operands = list(args)
        if partition_name is not None:
            operands.append(partition_id_tensor())
        return tuple(_bass_exec_p.bind(
            *operands,
            out_avals=tuple(out_avals),
            in_names=tuple(all_in_names),
            out_names=tuple(out_names),
            lowering_input_output_aliases=(),
            sim_require_finite=True,
            sim_require_nnan=True,
            nc=nc,
        ))

    devices = jax.devices()[:N_CORES]
    mesh = Mesh(np.asarray(devices), ("core",))
    in_specs = (PartitionSpec("core"),) * (n_params + len(out_names))
    out_specs = (PartitionSpec("core"),) * len(out_names)
    sharded = jax.jit(
        shard_map(_body, mesh=mesh, in_specs=in_specs, out_specs=out_specs,
                  check_rep=False),
        donate_argnums=donate, keep_unused=True)
    shard = NamedSharding(mesh, PartitionSpec("core"))
    v_dev = jax.device_put(
        np.concatenate([_v_const()] * N_CORES, axis=0), shard)
    _EXEC_CACHE = (sharded, in_names, out_names, zero_outs, shard, v_dev)
    return _EXEC_CACHE


def kernel(outputs, targets):
    """outputs, targets: [64, 128, 4] float32 -> scalar float32 loss."""
    sharded, in_names, out_names, zero_outs, shard, v_dev = _get_exec()
    outputs = np.asarray(outputs, np.float32)
    targets = np.asarray(targets, np.float32)
    B, T, C = outputs.shape
    t = np.transpose(targets, (0, 2, 1)).reshape(N_CORES, S, T)
    o = np.transpose(outputs, (0, 2, 1)).reshape(N_CORES, S, T)
    # per core: 64 partitions = [32 series | same 32 series again]
    t2 = np.ascontiguousarray(
        np.concatenate([t, t], axis=1).reshape(N_CORES * S2, T))
    o2 = np.ascontiguousarray(
        np.concatenate([o, o], axis=1).reshape(N_CORES * S2, T))
    by_name = {"t": t2, "o": o2, "v": v_dev}
    concat_in = [by_name[name] for name in in_names]
    concat_zeros = [
        np.zeros((N_CORES * z.shape[0], *z.shape[1:]), z.dtype) for z in zero_outs
    ]
    out_arrs = sharded(*concat_in, *concat_zeros)
    outs = {name: np.asarray(out_arrs[i]) for i, name in enumerate(out_names)}
    vv = outs["vals"].reshape(N_CORES, 2, S).astype(np.float64)
    A, Bm = vv[:, 0, :], vv[:, 1, :]
    vals = (A + Bm) / 2.0
    s_fd = (A - Bm) / (2.0 * EPS)
    loss = 0.5 * (vals.sum() / B) + 0.5 * (s_fd.sum() / (B * T * T))
    return np.float32(loss)


# revision 8
# speedup vs baseline: 2.1897x; 1.0216x over previous
"""DILATE loss (soft-DTW shape + temporal) on 8 Trainium2 NeuronCores.

Strategy: central finite difference. gamma=0.01 makes the soft-DTW
effectively a hard min-plus (Viterbi) DP, and the temporal term is
  sum(E * Omega) = d/d(eps) softdtw(D + eps*Omega)  at eps=0
since E = d(softdtw)/dD. So each series is solved TWICE — once on
D + eps*Omega and once on D - eps*Omega — and the host combines:
  vals = (A + B) / 2          (shape term)
  sum(E*Omega) = (A - B)/(2 eps)   (temporal term)
This removes the whole backward/suffix DP, the posterior epilogue
(+num, -D, exp/accumulate) and the -ln(Omega) fold of the previous
version; only ONE forward DP chain remains.

Layout per core: 32 series x {+eps, -eps} = 64 SBUF partitions; per-op
cost depends only on free-dim size, so the doubling is time-free.
  ACT : D rows fused build  Square(-o_j + t_i)  (bias = per-partition t_i)
  Pool: D += (+-eps)Omega   via tensor_tensor_scan(bypass, add) on flat
        chunks (the +- sign is baked into the per-half V constant)
  DVE : 128 x (TT-min + scan(min,add)) forward DP  — the critical path
"""
import sys
if "/opt/trn_rl_repo" not in sys.path:
    sys.path.insert(0, "/opt/trn_rl_repo")
import numpy as np
from contextlib import ExitStack

import concourse.bass as bass
import concourse.bacc as bacc
import concourse.mybir as mybir
import concourse.tile as tile
from concourse.mybir import AluOpType, ActivationFunctionType

F32 = mybir.dt.float32
F16 = mybir.dt.float16
S = 32          # series per core
S2 = 64         # partitions: series x {+eps, -eps}
N = 128         # DP size (= T)
RS = N + 1      # M-table row stride (col 0 = boundary)
BIG = 1e30
EPS = 3e-5      # FD step on the Omega perturbation
N_CORES = 8


def ap(t, off, dims):
    base = t[:]
    return bass.AP(base.tensor, base.offset + off, [base.ap[0]] + dims)


def _build_kernel():
    nc = bacc.Bacc("TRN2", target_bir_lowering=False, debug=False)
    t_d = nc.dram_tensor("t", [S2, N], F32, kind="ExternalInput")
    o_d = nc.dram_tensor("o", [S2, N], F32, kind="ExternalInput")
    v_d = nc.dram_tensor("v", [S2, N * N], F16, kind="ExternalInput")
    vals_d = nc.dram_tensor("vals", [S2, 1], F32, kind="ExternalOutput")

    with tile.TileContext(nc) as tc, ExitStack() as ctx:
        pool = ctx.enter_context(tc.tile_pool(name="main", bufs=1))
        t_s = pool.tile([S2, N], F32, tag="t_s")
        o_s = pool.tile([S2, N], F32, tag="o_s")
        v_s = pool.tile([S2, N * N], F16, tag="v_s")
        D_s = pool.tile([S2, N * N], F32, tag="D_s")
        M_s = pool.tile([S2, RS * RS], F32, tag="M_s")
        ent_s = pool.tile([S2, N], F32, tag="ent_s")
        vals_s = pool.tile([S2, 1], F32, tag="vals_s")

        # dummy activation on scratch: hoists the Square table load (1.28us)
        # to t=0 so it overlaps the input DMAs instead of stalling row 1
        nc.scalar.activation(vals_s[:], vals_s[:],
                             ActivationFunctionType.Square)

        nc.sync.dma_start(t_s[:], t_d.ap())
        nc.sync.dma_start(o_s[:], o_d.ap())
        # eps*Omega chunks: first ones small so the build pipeline starts fast
        vch = [2, 2, 4, 8, 16, 32, 64]
        r0 = 0
        for cn in vch:
            nc.sync.dma_start(
                ap(v_s, r0 * N, [[1, cn * N]]),
                bass.AP(v_d, r0 * N, [[N * N, S2], [1, cn * N]]))
            r0 += cn

        # M boundary: row 0 = BIG except M[0,0] = 0; col 0 of rows 1..N = BIG
        nc.gpsimd.memset(ap(M_s, 0, [[1, RS]]), BIG)
        nc.gpsimd.memset(ap(M_s, 0, [[1, 1]]), 0.0)
        nc.gpsimd.memset(ap(M_s, RS, [[RS, N], [1, 1]]), BIG)

        def m_off(r):
            return r * RS

        # D build: ACT fuses (t_i - o_j)^2 per row; Pool folds +-eps*Omega in
        # flat scan(bypass, add) chunks right behind it.
        # chunk boundaries grow ~geometrically (b <= 1.3a): the DP consumes
        # rows at ~388ns while ACT produces at ~292ns, so a chunk ending at
        # row b is only ready in time if the DP is past ~0.75b when it hits
        # the boundary
        row_chunks = [1, 1, 1, 1, 1, 1, 2, 2, 3, 4, 5, 7, 9, 12, 15, 20, 21, 22]
        r0 = 0
        for cn in row_chunks:
            for i in range(r0, r0 + cn):
                nc.scalar.activation(
                    ap(D_s, i * N, [[1, N]]), o_s[:],
                    ActivationFunctionType.Square,
                    bias=t_s[:, i:i + 1], scale=-1.0)
            dch = ap(D_s, r0 * N, [[1, cn * N]])
            nc.gpsimd.tensor_tensor_scan(
                dch, dch, ap(v_s, r0 * N, [[1, cn * N]]),
                0.0, AluOpType.bypass, AluOpType.add)
            r0 += cn

        # forward min-plus DP: M[r,j] = D[r,j] + min(M[r-1,j-1], M[r-1,j], M[r,j-1])
        dp_insts = []
        for r in range(1, N + 1):
            dp_insts.append(nc.vector.tensor_tensor(
                ent_s[:],
                ap(M_s, m_off(r - 1), [[1, N]]),
                ap(M_s, m_off(r - 1) + 1, [[1, N]]),
                AluOpType.min))
            dp_insts.append(nc.vector.tensor_tensor_scan(
                ap(M_s, m_off(r) + 1, [[1, N]]),
                ent_s[:],
                ap(D_s, (r - 1) * N, [[1, N]]),
                BIG, AluOpType.min, AluOpType.add))

        # The DP is one serial chain of same-engine (DVE) ops with
        # ascending same-shape access; engine in-order execution plus the
        # 128-cycle op length covers the SBUF write-ack pipeline, so the
        # intra-chain edges don't need runtime semaphores. Relax them to
        # nosync (queue-order) edges — without this every row pays two
        # ~90ns sem round-trips plus a SEQ-blocking EventSemaphore wait
        # (~290ns/row, +37us on the critical path).
        import bass_rust as _br
        _NOSYNC = _br.DependencyInfo.NO_SYNC_ONLY
        dp_names = {bi.ins.name for bi in dp_insts}
        for bi in dp_insts:
            inst = bi.ins
            for name, info in inst.dependency_edges():
                if name in dp_names:
                    inst.remove_dependency(name)
                    inst.add_dependency(name, _NOSYNC)

        nc.sync.dma_start(vals_d.ap(), ap(M_s, m_off(N) + N, [[1, 1]]))

    nc.compile()
    return nc


_NC_CACHE = None


def _get_nc():
    global _NC_CACHE
    if _NC_CACHE is None:
        _NC_CACHE = _build_kernel()
    return _NC_CACHE


def _v_const():
    """[S2, N*N] fp16: rows 0..31 = +eps*Omega, rows 32..63 = -eps*Omega."""
    idx = np.arange(N, dtype=np.float64)
    om = ((idx[:, None] - idx[None, :]) ** 2).reshape(-1)
    v = (EPS * om).astype(np.float16)
    return np.concatenate([
        np.broadcast_to(v, (S, N * N)),
        np.broadcast_to(-v, (S, N * N)),
    ]).astype(np.float16)


_EXEC_CACHE = None


def _get_exec():
    """Build the sharded jitted executable once (mirrors bass2jax's
    run_bass_via_pjrt multi-core path) and keep the constant v input
    resident on the devices."""
    global _EXEC_CACHE
    if _EXEC_CACHE is not None:
        return _EXEC_CACHE
    import jax
    import concourse.mybir as _mybir
    from jax.sharding import Mesh, PartitionSpec, NamedSharding
    from jax.experimental.shard_map import shard_map
    from concourse.bass2jax import (
        _bass_exec_p, install_neuronx_cc_hook, partition_id_tensor)

    nc = _get_nc()
    install_neuronx_cc_hook()
    partition_name = nc.partition_id_tensor.name if nc.partition_id_tensor else None
    in_names, out_names, out_avals, zero_outs = [], [], [], []
    for alloc in nc.m.functions[0].allocations:
        if not isinstance(alloc, _mybir.MemoryLocationSet):
            continue
        name = alloc.memorylocations[0].name
        if alloc.kind == "ExternalInput":
            if name != partition_name:
                in_names.append(name)
        elif alloc.kind == "ExternalOutput":
            shape = tuple(alloc.tensor_shape)
            dtype = _mybir.dt.np(alloc.dtype)
            out_names.append(name)
            out_avals.append(jax.core.ShapedArray(shape, dtype))
            zero_outs.append(np.zeros(shape, dtype))
    n_params = len(in_names)
    all_in_names = list(in_names) + list(out_names)
    if partition_name is not None:
        all_in_names.append(partition_name)
    donate = tuple(range(n_params, n_params + len(out_names)))

    def _body(*args):
        operands = list(args)
        if partition_name is not None:
            operands.append(partition_id_tensor())
        return tuple(_bass_exec_p.bind(
            *operands,
            out_avals=tuple(out_avals),
            in_names=tuple(all_in_names),
            out_names=tuple(out_names),
            lowering_input_output_aliases=(),
            sim_require_finite=True,
            sim_require_nnan=True,
            nc=nc,
        ))

    devices = jax.devices()[:N_CORES]
    mesh = Mesh(np.asarray(devices), ("core",))
    in_specs = (PartitionSpec("core"),) * (n_params + len(out_names))
    out_specs = (PartitionSpec("core"),) * len(out_names)
    sharded = jax.jit(
        shard_map(_body, mesh=mesh, in_specs=in_specs, out_specs=out_specs,
                  check_rep=False),
        donate_argnums=donate, keep_unused=True)
    shard = NamedSharding(mesh, PartitionSpec("core"))
    v_dev = jax.device_put(
        np.concatenate([_v_const()] * N_CORES, axis=0), shard)
    _EXEC_CACHE = (sharded, in_names, out_names, zero_outs, shard, v_dev)
    return _EXEC_CACHE


def kernel(outputs, targets):
    """outputs, targets: [64, 128, 4] float32 -> scalar float32 loss."""
    sharded, in_names, out_names, zero_outs, shard, v_dev = _get_exec()
    outputs = np.asarray(outputs, np.float32)
    targets = np.asarray(targets, np.float32)
    B, T, C = outputs.shape
    t = np.transpose(targets, (0, 2, 1)).reshape(N_CORES, S, T)
    o = np.transpose(outputs, (0, 2, 1)).reshape(N_CORES, S, T)
    # per core: 64 partitions = [32 series | same 32 series again]
    t2 = np.ascontiguousarray(
        np.concatenate([t, t], axis=1).reshape(N_CORES * S2, T))
    o2 = np.ascontiguousarray(
        np.concatenate([o, o], axis=1).reshape(N_CORES * S2, T))
    by_name = {"t": t2, "o": o2, "v": v_dev}
    concat_in = [by_name[name] for name in in_names]
    concat_zeros = [
        np.zeros((N_CORES * z.shape[0], *z.shape[1:]), z.dtype) for z in zero_outs
    ]
    out_arrs = sharded(*concat_in, *concat_zeros)
    outs = {name: np.asarray(out_arrs[i]) for i, name in enumerate(out_names)}
    vv = outs["vals"].reshape(N_CORES, 2, S).astype(np.float64)
    A, Bm = vv[:, 0, :], vv[:, 1, :]
    vals = (A + Bm) / 2.0
    s_fd = (A - Bm) / (2.0 * EPS)
    loss = 0.5 * (vals.sum() / B) + 0.5 * (s_fd.sum() / (B * T * T))
    return np.float32(loss)


# revision 12
# speedup vs baseline: 2.2571x; 1.0308x over previous
"""DILATE loss (soft-DTW shape + temporal) on 8 Trainium2 NeuronCores.

Strategy: central finite difference. gamma=0.01 makes the soft-DTW
effectively a hard min-plus (Viterbi) DP, and the temporal term is
  sum(E * Omega) = d/d(eps) softdtw(D + eps*Omega)  at eps=0
since E = d(softdtw)/dD. So each series is solved TWICE — once on
D + eps*Omega and once on D - eps*Omega — and the host combines:
  vals = (A + B) / 2          (shape term)
  sum(E*Omega) = (A - B)/(2 eps)   (temporal term)
This removes the whole backward/suffix DP, the posterior epilogue
(+num, -D, exp/accumulate) and the -ln(Omega) fold of the previous
version; only ONE forward DP chain remains.

Layout per core: 32 series x {+eps, -eps} = 64 SBUF partitions; per-op
cost depends only on free-dim size, so the doubling is time-free.
  ACT : D rows fused build  Square(-o_j + t_i)  (bias = per-partition t_i)
  Pool: D += (+-eps)Omega   via tensor_tensor_scan(bypass, add) on flat
        chunks (the +- sign is baked into the per-half V constant)
  DVE : 128 x (TT-min + scan(min,add)) forward DP  — the critical path
"""
import sys
if "/opt/trn_rl_repo" not in sys.path:
    sys.path.insert(0, "/opt/trn_rl_repo")
import numpy as np
from contextlib import ExitStack

import concourse.bass as bass
import concourse.bacc as bacc
import concourse.mybir as mybir
import concourse.tile as tile
from concourse.mybir import AluOpType, ActivationFunctionType

F32 = mybir.dt.float32
F16 = mybir.dt.float16
S = 32          # series per core
S2 = 64         # partitions: series x {+eps, -eps}
N = 128         # DP size (= T)
RS = N + 1      # M-table row stride (col 0 = boundary)
BIG = 1e30
EPS = 3e-5      # FD step on the Omega perturbation
N_CORES = 8


def ap(t, off, dims):
    base = t[:]
    return bass.AP(base.tensor, base.offset + off, [base.ap[0]] + dims)


def _build_kernel():
    nc = bacc.Bacc("TRN2", target_bir_lowering=False, debug=False)
    to_d = nc.dram_tensor("to", [S2, 2 * N], F32, kind="ExternalInput")
    v_d = nc.dram_tensor("v", [S2, N * N], F16, kind="ExternalInput")
    vals_d = nc.dram_tensor("vals", [S2, 1], F32, kind="ExternalOutput")

    with tile.TileContext(nc) as tc, ExitStack() as ctx:
        pool = ctx.enter_context(tc.tile_pool(name="main", bufs=1))
        to_s = pool.tile([S2, 2 * N], F32, tag="to_s")   # [t | o]
        v_s = pool.tile([S2, N * N], F16, tag="v_s")
        D_s = pool.tile([S2, N * N], F32, tag="D_s")
        M_s = pool.tile([S2, RS * RS], F32, tag="M_s")
        ent_s = pool.tile([S2, N], F32, tag="ent_s")
        scr_s = pool.tile([S2, 1], F32, tag="scr_s")

        # dummy activation on scratch: hoists the Square table load (1.28us)
        # to t=0 so it overlaps the input DMAs instead of stalling row 1
        nc.scalar.activation(scr_s[:], scr_s[:],
                             ActivationFunctionType.Square)

        nc.sync.dma_start(to_s[:], to_d.ap())
        # eps*Omega chunks: first ones small so the build pipeline starts fast
        vch = [2, 2, 4, 8, 16, 32, 64]
        r0 = 0
        for cn in vch:
            nc.sync.dma_start(
                ap(v_s, r0 * N, [[1, cn * N]]),
                bass.AP(v_d, r0 * N, [[N * N, S2], [1, cn * N]]))
            r0 += cn

        # M boundary: row 0 = BIG except M[0,0] = 0; col 0 of rows 1..N = BIG
        nc.gpsimd.memset(ap(M_s, 0, [[1, RS]]), BIG)
        nc.gpsimd.memset(ap(M_s, 0, [[1, 1]]), 0.0)
        nc.gpsimd.memset(ap(M_s, RS, [[RS, N], [1, 1]]), BIG)

        def m_off(r):
            return r * RS

        # D build: ACT fuses (t_i - o_j)^2 per row; +-eps*Omega is folded in
        # with scan(bypass, add). The first DVE_ADD_ROWS rows are added on
        # DVE itself (it is idle during startup and this keeps Pool's sem
        # hops off the early critical path); the rest go to Pool in chunks
        # whose boundaries grow ~20%/step — the DP consumes rows at ~388ns
        # while ACT produces at ~292ns and Pool adds ~178ns/row, so chunk
        # [a..b) is ready in time only if 0.292b + 0.178(b-a) stays below
        # 0.388a plus the starting lead.
        DVE_ADD_ROWS = 8
        dve_adds = []
        bounds = [DVE_ADD_ROWS]
        while bounds[-1] < N:
            a = bounds[-1]
            bounds.append(min(N, max(a + 1, int(1.18 * a - 0.4))))
        o_ap = ap(to_s, N, [[1, N]])
        for i in range(DVE_ADD_ROWS):
            nc.scalar.activation(
                ap(D_s, i * N, [[1, N]]), o_ap,
                ActivationFunctionType.Square,
                bias=ap(to_s, i, [[1, 1]]), scale=-1.0)
            dch = ap(D_s, i * N, [[1, N]])
            dve_adds.append(nc.vector.tensor_tensor_scan(
                dch, dch, ap(v_s, i * N, [[1, N]]),
                0.0, AluOpType.bypass, AluOpType.add))
        for a, b in zip(bounds[:-1], bounds[1:]):
            for i in range(a, b):
                nc.scalar.activation(
                    ap(D_s, i * N, [[1, N]]), o_ap,
                    ActivationFunctionType.Square,
                    bias=ap(to_s, i, [[1, 1]]), scale=-1.0)
            dch = ap(D_s, a * N, [[1, (b - a) * N]])
            nc.gpsimd.tensor_tensor_scan(
                dch, dch, ap(v_s, a * N, [[1, (b - a) * N]]),
                0.0, AluOpType.bypass, AluOpType.add)

        # forward min-plus DP: M[r,j] = D[r,j] + min(M[r-1,j-1], M[r-1,j], M[r,j-1])
        dp_insts = []
        for r in range(1, N + 1):
            dp_insts.append(nc.vector.tensor_tensor(
                ent_s[:],
                ap(M_s, m_off(r - 1), [[1, N]]),
                ap(M_s, m_off(r - 1) + 1, [[1, N]]),
                AluOpType.min))
            dp_insts.append(nc.vector.tensor_tensor_scan(
                ap(M_s, m_off(r) + 1, [[1, N]]),
                ent_s[:],
                ap(D_s, (r - 1) * N, [[1, N]]),
                BIG, AluOpType.min, AluOpType.add))

        # The DP is one serial chain of same-engine (DVE) ops with
        # ascending same-shape access; engine in-order execution plus the
        # 128-cycle op length covers the SBUF write-ack pipeline, so the
        # intra-chain edges don't need runtime semaphores. Relax them to
        # nosync (queue-order) edges — without this every row pays two
        # ~90ns sem round-trips plus a SEQ-blocking EventSemaphore wait
        # (~290ns/row, +37us on the critical path).
        import bass_rust as _br
        _NOSYNC = _br.DependencyInfo.NO_SYNC_ONLY
        dp_insts = dve_adds + dp_insts
        dp_names = {bi.ins.name for bi in dp_insts}
        for bi in dp_insts:
            inst = bi.ins
            for name, info in inst.dependency_edges():
                if name in dp_names:
                    inst.remove_dependency(name)
                    inst.add_dependency(name, _NOSYNC)

        nc.sync.dma_start(vals_d.ap(), ap(M_s, m_off(N) + N, [[1, 1]]))

    nc.compile()
    return nc


_NC_CACHE = None


def _get_nc():
    global _NC_CACHE
    if _NC_CACHE is None:
        _NC_CACHE = _build_kernel()
    return _NC_CACHE


def _v_const():
    """[S2, N*N] fp16: rows 0..31 = +eps*Omega, rows 32..63 = -eps*Omega."""
    idx = np.arange(N, dtype=np.float64)
    om = ((idx[:, None] - idx[None, :]) ** 2).reshape(-1)
    v = (EPS * om).astype(np.float16)
    return np.concatenate([
        np.broadcast_to(v, (S, N * N)),
        np.broadcast_to(-v, (S, N * N)),
    ]).astype(np.float16)


_EXEC_CACHE = None


def _get_exec():
    """Build the sharded jitted executable once (mirrors bass2jax's
    run_bass_via_pjrt multi-core path) and keep the constant v input
    resident on the devices."""
    global _EXEC_CACHE
    if _EXEC_CACHE is not None:
        return _EXEC_CACHE
    import jax
    import concourse.mybir as _mybir
    from jax.sharding import Mesh, PartitionSpec, NamedSharding
    from jax.experimental.shard_map import shard_map
    from concourse.bass2jax import (
        _bass_exec_p, install_neuronx_cc_hook, partition_id_tensor)

    nc = _get_nc()
    install_neuronx_cc_hook()
    partition_name = nc.partition_id_tensor.name if nc.partition_id_tensor else None
    in_names, out_names, out_avals, zero_outs = [], [], [], []
    for alloc in nc.m.functions[0].allocations:
        if not isinstance(alloc, _mybir.MemoryLocationSet):
            continue
        name = alloc.memorylocations[0].name
        if alloc.kind == "ExternalInput":
            if name != partition_name:
                in_names.append(name)
        elif alloc.kind == "ExternalOutput":
            shape = tuple(alloc.tensor_shape)
            dtype = _mybir.dt.np(alloc.dtype)
            out_names.append(name)
            out_avals.append(jax.core.ShapedArray(shape, dtype))
            zero_outs.append(np.zeros(shape, dtype))
    n_params = len(in_names)
    all_in_names = list(in_names) + list(out_names)
    if partition_name is not None:
        all_in_names.append(partition_name)
    donate = tuple(range(n_params, n_params + len(out_names)))

    def _body(*args):
        operands = list(args)
        if partition_name is not None:
            operands.append(partition_id_tensor())
        return tuple(_bass_exec_p.bind(
            *operands,
            out_avals=tuple(out_avals),
            in_names=tuple(all_in_names),
            out_names=tuple(out_names),
            lowering_input_output_aliases=(),
            sim_require_finite=True,
            sim_require_nnan=True,
            nc=nc,
        ))

    devices = jax.devices()[:N_CORES]
    mesh = Mesh(np.asarray(devices), ("core",))
    in_specs = (PartitionSpec("core"),) * (n_params + len(out_names))
    out_specs = (PartitionSpec("core"),) * len(out_names)
    sharded = jax.jit(
        shard_map(_body, mesh=mesh, in_specs=in_specs, out_specs=out_specs,
                  check_rep=False),
        donate_argnums=donate, keep_unused=True)
    shard = NamedSharding(mesh, PartitionSpec("core"))
    v_dev = jax.device_put(
        np.concatenate([_v_const()] * N_CORES, axis=0), shard)
    _EXEC_CACHE = (sharded, in_names, out_names, zero_outs, shard, v_dev)
    return _EXEC_CACHE


def kernel(outputs, targets):
    """outputs, targets: [64, 128, 4] float32 -> scalar float32 loss."""
    sharded, in_names, out_names, zero_outs, shard, v_dev = _get_exec()
    outputs = np.asarray(outputs, np.float32)
    targets = np.asarray(targets, np.float32)
    B, T, C = outputs.shape
    t = np.transpose(targets, (0, 2, 1)).reshape(N_CORES, S, T)
    o = np.transpose(outputs, (0, 2, 1)).reshape(N_CORES, S, T)
    # per core: 64 partitions = [32 series | same 32 series again],
    # free dim = [t | o]
    to = np.concatenate([t, o], axis=2)                      # [8, 32, 2T]
    to2 = np.ascontiguousarray(
        np.concatenate([to, to], axis=1).reshape(N_CORES * S2, 2 * T))
    by_name = {"to": to2, "v": v_dev}
    concat_in = [by_name[name] for name in in_names]
    concat_zeros = [
        np.zeros((N_CORES * z.shape[0], *z.shape[1:]), z.dtype) for z in zero_outs
    ]
    out_arrs = sharded(*concat_in, *concat_zeros)
    outs = {name: np.asarray(out_arrs[i]) for i, name in enumerate(out_names)}
    vv = outs["vals"].reshape(N_CORES, 2, S).astype(np.float64)
    A, Bm = vv[:, 0, :], vv[:, 1, :]
    vals = (A + Bm) / 2.0
    s_fd = (A - Bm) / (2.0 * EPS)
    loss = 0.5 * (vals.sum() / B) + 0.5 * (s_fd.sum() / (B * T * T))
    return np.float32(loss)
